# revision 2
# baseline (speedup 1.0000x reference)
"""Trainium2 Bass kernel for nn_KolmogorovLayer (dense_mlp).

Math (reference):
    h   = tanh(x[:,:,None] * W1 + b1)              # [B, D, I]
    psi = einsum('bdi,dio->bdo', h, W2) + b2       # [B, D, I]
    hg  = tanh(psi.reshape(B, D*I) @ Wg1 + bg1)    # [B, I]
    out = hg @ Wg2 + bg2                           # [B, 1]

Because psi feeds a linear layer, fold W2 and Wg1 on the host:
    Meff[(d,i), f] = sum_o W2[d,i,o] * Wg1[d*I+o, f]
    beff[f]        = bg1[f] + b2.reshape(-1) @ Wg1
    u = h.reshape(B, D*I) @ Meff + beff;  out = tanh(u) @ Wg2 + bg2
This halves the matmul FLOPs. The kernel then runs a transposed pipeline
per core (batch sharded 8 ways, 4096 rows/core):
    - DMA-transpose x -> xT [64, 4096] in SBUF
    - per 128-row (d,i)-chunk: a K=2 selection matmul broadcasts the two
      needed xT rows onto 128 PSUM partitions; ScalarE computes
      h = tanh(W1*z + b1) with per-partition scale/bias (fused for free)
    - main contraction accumulates u in PSUM over 32 chunks (fp32r matmuls)
    - ScalarE tanh(u + beff), PE matvec with Wg2, +bg2 on VectorE, DMA out.
"""

import numpy as np
from contextlib import ExitStack

import concourse.bass as bass
import concourse.bacc as bacc
import concourse.mybir as mybir
import concourse.tile as tile
from concourse.bass import ts, ds
from concourse.bass_utils import run_bass_kernel_spmd
from concourse.masks import make_identity

F32 = mybir.dt.float32
F32R = mybir.dt.float32r

B_TOT, D, I = 32768, 64, 64
N_CORES = 8
BS = B_TOT // N_CORES          # 4096 rows per core
CHUNKS = D * I // 128          # 32 chunks of 128 (d,i) rows
W = 1024                       # batch window (PSUM-limited)
NW = BS // W                   # 4 windows
NS = 512                       # matmul moving-operand stream width


def _build_program(reps: int = 1):
    nc = bacc.Bacc(
        "TRN2", target_bir_lowering=False, debug=False, num_devices=N_CORES
    )
    x_d = nc.dram_tensor("x", [BS, D], F32, kind="ExternalInput").ap()
    w1_d = nc.dram_tensor("w1c", [128, CHUNKS], F32, kind="ExternalInput").ap()
    b1_d = nc.dram_tensor("b1c", [128, CHUNKS], F32, kind="ExternalInput").ap()
    me_d = nc.dram_tensor("meffc", [128, CHUNKS * I], F32R, kind="ExternalInput").ap()
    be_d = nc.dram_tensor("beff", [I, 1], F32, kind="ExternalInput").ap()
    wg_d = nc.dram_tensor("wg2", [I, 1], F32R, kind="ExternalInput").ap()
    sel_d = nc.dram_tensor("sel", [I, CHUNKS * 128], F32R, kind="ExternalInput").ap()
    bg_d = nc.dram_tensor("bg2", [1, 1], F32, kind="ExternalInput").ap()
    y_d = nc.dram_tensor("y", [BS, 1], F32, kind="ExternalOutput").ap()
    y_row = y_d.rearrange("b one -> one b")

    # x viewed so one DMA lands [128, 8, 64] per window:
    # xbm[p, t, d] = x[t*128 + p, d]
    x_bm = x_d.rearrange("(t p) d -> p t d", p=128)

    with tile.TileContext(nc) as tc, ExitStack() as ctx:
        const = ctx.enter_context(tc.tile_pool(name="const", bufs=1))
        hpool = ctx.enter_context(tc.tile_pool(name="h", bufs=3))
        hgpool = ctx.enter_context(tc.tile_pool(name="hg", bufs=2))
        opool = ctx.enter_context(tc.tile_pool(name="osb", bufs=2))
        zpool = ctx.enter_context(tc.tile_pool(name="z", bufs=2, space="PSUM"))
        ypool = ctx.enter_context(tc.tile_pool(name="yps", bufs=1, space="PSUM"))
        vpool = ctx.enter_context(tc.tile_pool(name="vps", bufs=1, space="PSUM"))
        tpool = ctx.enter_context(tc.tile_pool(name="tps", bufs=1, space="PSUM"))

        xT = const.tile([D, BS], F32R)
        xbm = const.tile([128, CHUNKS * D], F32)
        ident = const.tile([128, 128], F32)
        w1s = const.tile([128, CHUNKS], F32)
        b1s = const.tile([128, CHUNKS], F32)
        mes = const.tile([128, CHUNKS * I], F32R)
        bes = const.tile([I, 1], F32)
        wgs = const.tile([I, 1], F32R)
        sls = const.tile([I, CHUNKS * 128], F32R)
        bgs = const.tile([1, 1], F32)

        make_identity(nc, ident[:])
        nc.sync.dma_start(w1s[:], w1_d)
        nc.sync.dma_start(b1s[:], b1_d)
        nc.sync.dma_start(mes[:], me_d)
        nc.sync.dma_start(bes[:], be_d)
        nc.sync.dma_start(wgs[:], wg_d)
        nc.sync.dma_start(sls[:], sel_d)
        nc.sync.dma_start(bgs[:], bg_d)

        if reps > 1:
            loop_ctx = tc.For_i(0, reps, 1)
            loop_ctx.__enter__()
        for w in range(NW):
            # stage x window into SBUF batch-major, then PE-transpose to xT
            nc.sync.dma_start(
                xbm[:, ts(w, 512)].rearrange("p (t d) -> p t d", d=D),
                x_bm[:, ts(w, 8), :],
            )
            for h2 in range(2):
                tp = tpool.tile([D, 512], F32)
                for j in range(4):
                    nc.tensor.transpose(
                        tp[:, ts(j, 128)],
                        xbm[:, ds(w * 512 + (4 * h2 + j) * 64, 64)],
                        ident[:],
                    )
                nc.vector.tensor_copy(
                    xT[:, ds(w * W + h2 * 512, 512)], tp[:]
                )

        # Software-pipelined main loop over 128 flattened (window, chunk)
        # steps.  The selection matmul for step j+2 is emitted AFTER step j's
        # main matmul, so in PE program order the PE never sits behind the
        # ScalarE tanh of the current chunk: steady state is
        #   PE:      ... main_j, sel_{j+2} ...   (854 ns/step)
        #   ScalarE: ... act_{j+1} ...           (1038 ns/step, bottleneck)
        steps = [(w, c) for w in range(NW) for c in range(CHUNKS)]
        LOOKAHEAD = 2
        ztiles = {}

        def emit_sel(j):
            w, c = steps[j]
            z = zpool.tile([128, W], F32)
            ztiles[j] = z
            for s in range(W // NS):
                nc.tensor.matmul(
                    z[:, ts(s, NS)],
                    sls[:, ts(c, 128)],
                    xT[:, ds(w * W + s * NS, NS)],
                    start=True,
                    stop=True,
                )

        for j in range(LOOKAHEAD):
            emit_sel(j)

        ypsum = None
        for j, (w, c) in enumerate(steps):
            if c == 0:
                ypsum = ypool.tile([I, W], F32)
            z = ztiles.pop(j)
            h = hpool.tile([128, W], F32R)
            nc.scalar.activation(
                h[:],
                z[:],
                mybir.ActivationFunctionType.Tanh,
                bias=b1s[:, c : c + 1],
                scale=w1s[:, c : c + 1],
            )
            for s in range(W // NS):
                nc.tensor.matmul(
                    ypsum[:, ts(s, NS)],
                    mes[:, ts(c, I)],
                    h[:, ts(s, NS)],
                    start=(c == 0),
                    stop=(c == CHUNKS - 1),
                )
            if j + LOOKAHEAD < len(steps):
                emit_sel(j + LOOKAHEAD)
            if c == CHUNKS - 1:
                hg = hgpool.tile([I, W], F32R)
                nc.scalar.activation(
                    hg[:],
                    ypsum[:],
                    mybir.ActivationFunctionType.Tanh,
                    bias=bes[:, 0:1],
                )
                osb = opool.tile([1, W], F32)
                for s in range(W // NS):
                    ops = vpool.tile([1, NS], F32)
                    nc.tensor.matmul(
                        ops[:],
                        wgs[:],
                        hg[:, ts(s, NS)],
                        start=True,
                        stop=True,
                    )
                    nc.vector.tensor_scalar_add(
                        osb[:, ts(s, NS)], ops[:], bgs[0:1, 0:1]
                    )
                nc.sync.dma_start(y_row[:, ts(w, W)], osb[:])
        if reps > 1:
            loop_ctx.__exit__(None, None, None)

    nc.compile()
    return nc


_PROGRAM_CACHE = {}


def _get_program(reps: int = 1):
    if reps not in _PROGRAM_CACHE:
        _PROGRAM_CACHE[reps] = _build_program(reps)
    return _PROGRAM_CACHE[reps]


def _round_f32r(a):
    """Round fp32 to the nearest value representable as bf16_hi + bf16_lo."""
    import ml_dtypes
    a = np.asarray(a, np.float32)
    hi = a.astype(ml_dtypes.bfloat16).astype(np.float32)
    lo = (a - hi).astype(ml_dtypes.bfloat16).astype(np.float32)
    return hi + lo


def _prepare_weight_maps(W1, b1, W2, b2, Wg1, bg1, Wg2, bg2):
    W1 = np.asarray(W1, np.float32)
    b1 = np.asarray(b1, np.float32)
    W2 = np.asarray(W2, np.float64)
    b2 = np.asarray(b2, np.float64)
    Wg1 = np.asarray(Wg1, np.float64)
    bg1 = np.asarray(bg1, np.float64)
    Wg2 = np.asarray(Wg2, np.float32)
    bg2 = np.asarray(bg2, np.float32)

    # Fold: Meff[(d,i), f] = sum_o W2[d,i,o] Wg1[d*I+o, f]
    Wg1r = Wg1.reshape(D, I, I)
    Meff = np.einsum("dio,dof->dif", W2, Wg1r).astype(np.float32)  # [D, I, I]
    beff = (bg1 + b2.reshape(-1) @ Wg1).astype(np.float32)  # [I]

    # Chunk layouts: chunk c covers d = 2c, 2c+1; partition p = (d_rel<<6)|i
    # w1c[p, c] = W1[2c + (p>>6), p & 63]
    w1c = W1.reshape(CHUNKS, 2 * I).T.copy()  # [128, 32]
    b1c = b1.astype(np.float32).reshape(CHUNKS, 2 * I).T.copy()
    # meffc[p, c*I + f] = Meff[2c + (p>>6), p&63, f]
    meffc = (
        Meff.reshape(CHUNKS, 2 * I, I).transpose(1, 0, 2).reshape(128, CHUNKS * I)
    ).copy()
    # sel[k, c*128 + m] = 1 where k == 2c + (m>>6): routes xT row d=2c+(m>>6)
    # to chunk-partition m
    sel = np.zeros((I, CHUNKS, 128), np.float32)
    for c in range(CHUNKS):
        sel[2 * c, c, 0:I] = 1.0
        sel[2 * c + 1, c, I:128] = 1.0
    sel = sel.reshape(I, CHUNKS * 128)
    return {
        "w1c": np.ascontiguousarray(w1c, np.float32),
        "b1c": np.ascontiguousarray(b1c, np.float32),
        "meffc": _round_f32r(np.ascontiguousarray(meffc, np.float32)),
        "beff": beff.reshape(I, 1).copy(),
        "wg2": _round_f32r(Wg2.reshape(I, 1)),
        "sel": sel,
        "bg2": bg2.reshape(1, 1).astype(np.float32).copy(),
    }


def kernel(x, W1, b1, W2, b2, Wg1, bg1, Wg2, bg2, _trace=False):
    x = np.ascontiguousarray(np.asarray(x, np.float32))
    assert x.shape == (B_TOT, D)
    wmap = _prepare_weight_maps(W1, b1, W2, b2, Wg1, bg1, Wg2, bg2)
    nc = _get_program()
    in_maps = [
        {"x": np.ascontiguousarray(x[i * BS : (i + 1) * BS]), **wmap}
        for i in range(N_CORES)
    ]
    res = run_bass_kernel_spmd(nc, in_maps, list(range(N_CORES)), trace=_trace)
    y = np.concatenate([r["y"] for r in res.results], axis=0)
    if _trace:
        kernel.last_results = res
    return y.astype(np.float32)



# revision 9
# speedup vs baseline: 1.7218x; 1.7218x over previous
"""Trainium2 Bass kernel for nn_KolmogorovLayer (dense_mlp).

Math (reference):
    h   = tanh(x[:,:,None] * W1 + b1)              # [B, D, I]
    psi = einsum('bdi,dio->bdo', h, W2) + b2       # [B, D, I]
    hg  = tanh(psi.reshape(B, D*I) @ Wg1 + bg1)    # [B, I]
    out = hg @ Wg2 + bg2                           # [B, 1]

Because psi feeds a linear layer, fold W2 and Wg1 on the host:
    Meff[(d,i), f] = sum_o W2[d,i,o] * Wg1[d*I+o, f]
    beff[f]        = bg1[f] + b2.reshape(-1) @ Wg1
    u = h.reshape(B, D*I) @ Meff + beff;  out = tanh(u) @ Wg2 + bg2
This halves the matmul FLOPs. The kernel then runs a transposed pipeline
per core (batch sharded 8 ways, 4096 rows/core):
    - DMA-transpose x -> xT [64, 4096] in SBUF
    - per 128-row (d,i)-chunk: a K=2 selection matmul broadcasts the two
      needed xT rows onto 128 PSUM partitions; ScalarE computes
      h = tanh(W1*z + b1) with per-partition scale/bias (fused for free)
    - main contraction accumulates u in PSUM over 32 chunks (fp32r matmuls)
    - ScalarE tanh(u + beff), PE matvec with Wg2, +bg2 on VectorE, DMA out.
"""

import numpy as np
from contextlib import ExitStack

import concourse.bass as bass
import concourse.bacc as bacc
import concourse.mybir as mybir
import concourse.tile as tile
from concourse.bass import ts, ds
from concourse.bass_utils import run_bass_kernel_spmd
from concourse.masks import make_identity

F32 = mybir.dt.float32
F32R = mybir.dt.float32r

B_TOT, D, I = 32768, 64, 64
N_CORES = 8
BS = B_TOT // N_CORES          # 4096 rows per core
CHUNKS = D * I // 128          # 32 chunks of 128 (d,i) rows
W = 1024                       # batch window (PSUM-limited)
NW = BS // W                   # 4 windows
NS = 512                       # matmul moving-operand stream width


def _build_program(reps: int = 1):
    nc = bacc.Bacc(
        "TRN2", target_bir_lowering=False, debug=False, num_devices=N_CORES
    )
    x_d = nc.dram_tensor("x", [BS, D], F32, kind="ExternalInput").ap()
    me_d = nc.dram_tensor("meffc", [128, CHUNKS * I], F32R, kind="ExternalInput").ap()
    be_d = nc.dram_tensor("beff", [I, 1], F32, kind="ExternalInput").ap()
    wg_d = nc.dram_tensor("wg2", [I, 1], F32R, kind="ExternalInput").ap()
    # selection matrix with the psi first-layer affine folded in:
    # row k<64 holds W1-scaled routing, row 64 holds b1 (paired with the
    # constant-ones row of xTa), so z = W1*x + b1 comes out of the PE and the
    # ScalarE activation is a pure tanh with no per-partition scale/bias.
    sel_d = nc.dram_tensor("sel", [I + 1, CHUNKS * 128], F32R, kind="ExternalInput").ap()
    bg_d = nc.dram_tensor("bg2", [1, 1], F32, kind="ExternalInput").ap()
    y_d = nc.dram_tensor("y", [BS, 1], F32, kind="ExternalOutput").ap()
    y_row = y_d.rearrange("b one -> one b")

    # x viewed so one DMA lands [128, 8, 64] per window:
    # xbm[p, t, d] = x[t*128 + p, d]
    x_bm = x_d.rearrange("(t p) d -> p t d", p=128)

    with tile.TileContext(nc) as tc, ExitStack() as ctx:
        const = ctx.enter_context(tc.tile_pool(name="const", bufs=1))
        hpool = ctx.enter_context(tc.tile_pool(name="h", bufs=3))
        hgpool = ctx.enter_context(tc.tile_pool(name="hg", bufs=2))
        opool = ctx.enter_context(tc.tile_pool(name="osb", bufs=2))
        zpool = ctx.enter_context(tc.tile_pool(name="z", bufs=2, space="PSUM"))
        ypool = ctx.enter_context(tc.tile_pool(name="yps", bufs=1, space="PSUM"))
        vpool = ctx.enter_context(tc.tile_pool(name="vps", bufs=1, space="PSUM"))
        tpool = ctx.enter_context(tc.tile_pool(name="tps", bufs=1, space="PSUM"))

        xT = const.tile([D + 1, BS], F32R)
        xbm = const.tile([128, CHUNKS * D], F32)
        ident = const.tile([128, 128], F32)
        mes = const.tile([128, CHUNKS * I], F32R)
        bes = const.tile([I, 1], F32)
        wgs = const.tile([I, 1], F32R)
        sls = const.tile([I + 1, CHUNKS * 128], F32R)
        bgs = const.tile([1, 1], F32)

        make_identity(nc, ident[:])
        nc.vector.memset(xT[D : D + 1, :].bitcast(F32), 1.0)
        nc.sync.dma_start(mes[:], me_d)
        nc.sync.dma_start(bes[:], be_d)
        nc.sync.dma_start(wgs[:], wg_d)
        nc.sync.dma_start(sls[:], sel_d)
        nc.sync.dma_start(bgs[:], bg_d)

        if reps > 1:
            loop_ctx = tc.For_i(0, reps, 1)
            loop_ctx.__enter__()
        for w in range(NW):
            # stage x window into SBUF batch-major, then PE-transpose to xT
            nc.sync.dma_start(
                xbm[:, ts(w, 512)].rearrange("p (t d) -> p t d", d=D),
                x_bm[:, ts(w, 8), :],
            )
            for h2 in range(2):
                tp = tpool.tile([D, 512], F32)
                for j in range(4):
                    nc.tensor.transpose(
                        tp[:, ts(j, 128)],
                        xbm[:, ds(w * 512 + (4 * h2 + j) * 64, 64)],
                        ident[:],
                    )
                nc.vector.tensor_copy(
                    xT[0:D, ds(w * W + h2 * 512, 512)], tp[:]
                )

        # Software-pipelined main loop over 128 flattened (window, chunk)
        # steps.  The selection matmul for step j+2 is emitted AFTER step j's
        # main matmul, so in PE program order the PE never sits behind the
        # ScalarE tanh of the current chunk: steady state is
        #   PE:      ... main_j, sel_{j+2} ...   (854 ns/step)
        #   ScalarE: ... act_{j+1} ...           (1038 ns/step, bottleneck)
        steps = [(w, c) for w in range(NW) for c in range(CHUNKS)]
        LOOKAHEAD = 2
        ztiles = {}

        def emit_sel(j):
            w, c = steps[j]
            z = zpool.tile([128, W], F32)
            ztiles[j] = z
            for s in range(W // NS):
                nc.tensor.matmul(
                    z[:, ts(s, NS)],
                    sls[:, ts(c, 128)],
                    xT[:, ds(w * W + s * NS, NS)],
                    start=True,
                    stop=True,
                )

        def emit_act(j, z, h):
            # pure tanh: z already carries W1*x + b1 from the selection matmul
            nc.scalar.activation(h[:], z[:], mybir.ActivationFunctionType.Tanh)

        for j in range(LOOKAHEAD):
            emit_sel(j)

        ypsum = None
        for j, (w, c) in enumerate(steps):
            if c == 0:
                ypsum = ypool.tile([I, W], F32)
            z = ztiles.pop(j)
            h = hpool.tile([128, W], F32R)
            emit_act(j, z, h)
            for s in range(W // NS):
                nc.tensor.matmul(
                    ypsum[:, ts(s, NS)],
                    mes[:, ts(c, I)],
                    h[:, ts(s, NS)],
                    start=(c == 0),
                    stop=(c == CHUNKS - 1),
                )
            if j + LOOKAHEAD < len(steps):
                emit_sel(j + LOOKAHEAD)
            if c == CHUNKS - 1:
                hg = hgpool.tile([I, W], F32R)
                nc.scalar.activation(
                    hg[:],
                    ypsum[:],
                    mybir.ActivationFunctionType.Tanh,
                    bias=bes[:, 0:1],
                )
                osb = opool.tile([1, W], F32)
                for s in range(W // NS):
                    ops = vpool.tile([1, NS], F32)
                    nc.tensor.matmul(
                        ops[:],
                        wgs[:],
                        hg[:, ts(s, NS)],
                        start=True,
                        stop=True,
                    )
                    nc.vector.tensor_scalar_add(
                        osb[:, ts(s, NS)], ops[:], bgs[0:1, 0:1]
                    )
                nc.sync.dma_start(y_row[:, ts(w, W)], osb[:])
        if reps > 1:
            loop_ctx.__exit__(None, None, None)

    nc.compile()
    return nc


_PROGRAM_CACHE = {}


def _get_program(reps: int = 1):
    if reps not in _PROGRAM_CACHE:
        _PROGRAM_CACHE[reps] = _build_program(reps)
    return _PROGRAM_CACHE[reps]


def _round_f32r(a):
    """Round fp32 to the nearest value representable as bf16_hi + bf16_lo."""
    import ml_dtypes
    a = np.asarray(a, np.float32)
    hi = a.astype(ml_dtypes.bfloat16).astype(np.float32)
    lo = (a - hi).astype(ml_dtypes.bfloat16).astype(np.float32)
    return hi + lo


def _prepare_weight_maps(W1, b1, W2, b2, Wg1, bg1, Wg2, bg2):
    W1 = np.asarray(W1, np.float32)
    b1 = np.asarray(b1, np.float32)
    W2 = np.asarray(W2, np.float64)
    b2 = np.asarray(b2, np.float64)
    Wg1 = np.asarray(Wg1, np.float64)
    bg1 = np.asarray(bg1, np.float64)
    Wg2 = np.asarray(Wg2, np.float32)
    bg2 = np.asarray(bg2, np.float32)

    # Fold: Meff[(d,i), f] = sum_o W2[d,i,o] Wg1[d*I+o, f]
    Wg1r = Wg1.reshape(D, I, I)
    Meff = np.einsum("dio,dof->dif", W2, Wg1r).astype(np.float32)  # [D, I, I]
    beff = (bg1 + b2.reshape(-1) @ Wg1).astype(np.float32)  # [I]

    # Chunk layouts: chunk c covers d = 2c, 2c+1; partition p = (d_rel<<6)|i
    # meffc[p, c*I + f] = Meff[2c + (p>>6), p&63, f]
    meffc = (
        Meff.reshape(CHUNKS, 2 * I, I).transpose(1, 0, 2).reshape(128, CHUNKS * I)
    ).copy()
    # sel[k, c*128 + m]: k<64 routes xT row d=2c+(m>>6) to chunk-partition m,
    # scaled by W1[d, i(m)]; row 64 pairs with the ones-row of xTa and adds
    # b1[d, i(m)], so z = W1*x + b1 directly from the PE.
    W1f = np.asarray(W1, np.float32)
    b1f = np.asarray(b1, np.float32)
    sel = np.zeros((I + 1, CHUNKS, 128), np.float32)
    for c in range(CHUNKS):
        sel[2 * c, c, 0:I] = W1f[2 * c]
        sel[2 * c + 1, c, I:128] = W1f[2 * c + 1]
        sel[I, c, 0:I] = b1f[2 * c]
        sel[I, c, I:128] = b1f[2 * c + 1]
    sel = sel.reshape(I + 1, CHUNKS * 128)
    return {
        "meffc": _round_f32r(np.ascontiguousarray(meffc, np.float32)),
        "beff": beff.reshape(I, 1).copy(),
        "wg2": _round_f32r(Wg2.reshape(I, 1)),
        "sel": _round_f32r(sel),
        "bg2": bg2.reshape(1, 1).astype(np.float32).copy(),
    }


def kernel(x, W1, b1, W2, b2, Wg1, bg1, Wg2, bg2, _trace=False):
    x = np.ascontiguousarray(np.asarray(x, np.float32))
    assert x.shape == (B_TOT, D)
    wmap = _prepare_weight_maps(W1, b1, W2, b2, Wg1, bg1, Wg2, bg2)
    nc = _get_program()
    in_maps = [
        {"x": np.ascontiguousarray(x[i * BS : (i + 1) * BS]), **wmap}
        for i in range(N_CORES)
    ]
    res = run_bass_kernel_spmd(nc, in_maps, list(range(N_CORES)), trace=_trace)
    y = np.concatenate([r["y"] for r in res.results], axis=0)
    if _trace:
        kernel.last_results = res
    return y.astype(np.float32)



# revision 17
# speedup vs baseline: 1.9395x; 1.1264x over previous
"""Trainium2 Bass kernel for nn_KolmogorovLayer (dense_mlp).

Math (reference):
    h   = tanh(x[:,:,None] * W1 + b1)              # [B, D, I]
    psi = einsum('bdi,dio->bdo', h, W2) + b2       # [B, D, I]
    hg  = tanh(psi.reshape(B, D*I) @ Wg1 + bg1)    # [B, I]
    out = hg @ Wg2 + bg2                           # [B, 1]

Because psi feeds a linear layer, fold W2 and Wg1 on the host:
    Meff[(d,i), f] = sum_o W2[d,i,o] * Wg1[d*I+o, f]
    beff[f]        = bg1[f] + b2.reshape(-1) @ Wg1
    u = h.reshape(B, D*I) @ Meff + beff;  out = tanh(u) @ Wg2 + bg2

The kernel runs a transposed pipeline per core (batch sharded 8 ways,
4096 rows/core), with EVERY affine folded into PE matmuls so that all
ScalarE activations are pure tanh (vector scale/bias operands cost
~1.2us per activation on real HW):
    - DMA x -> SBUF batch-major, PE-transpose to xT [64, 4096]; xT row 64
      is a constant-ones row.
    - per 128-row (d,i)-chunk: a K=65 selection matmul computes
      z = W1*x + b1 directly in PSUM (W1 in the selection rows, b1 against
      the ones row); ScalarE computes h = tanh(z), pure.
    - main contraction accumulates u in PSUM over 32 chunks; beff enters
      via a [1,64] x ones matmul that opens the accumulation group.
    - ScalarE tanh(u) -> hg (rows 0..63 of a tile whose row 64 is ones);
      a K=65 matvec with [Wg2; bg2] gives y in PSUM; DMA out from PSUM.

Engine budget per core per iteration (cost-model):  ScalarE 132x1038ns
(bottleneck), PE ~113us, PSUM 8 banks = z 3x2 + ypsum 2.
"""

import numpy as np
from contextlib import ExitStack

import concourse.bass as bass
import concourse.bacc as bacc
import concourse.mybir as mybir
import concourse.tile as tile
from concourse.bass import ts, ds
from concourse.bass_utils import run_bass_kernel_spmd
from concourse.masks import make_identity

F32 = mybir.dt.float32
F32R = mybir.dt.float32r

B_TOT, D, I = 32768, 64, 64
N_CORES = 8
BS = B_TOT // N_CORES          # 4096 rows per core
CHUNKS = D * I // 128          # 32 chunks of 128 (d,i) rows
W = 1024                       # batch window (PSUM-limited)
NW = BS // W                   # 4 windows
NS = 512                       # matmul moving-operand stream width


def _build_program(reps: int = 1):
    nc = bacc.Bacc(
        "TRN2", target_bir_lowering=False, debug=False, num_devices=N_CORES
    )
    x_d = nc.dram_tensor("x", [BS, D], F32, kind="ExternalInput").ap()
    me_d = nc.dram_tensor("meffc", [128, CHUNKS * I], F32R, kind="ExternalInput").ap()
    be_d = nc.dram_tensor("befr", [1, I], F32R, kind="ExternalInput").ap()
    wg_d = nc.dram_tensor("wgb", [I + 1, 1], F32R, kind="ExternalInput").ap()
    # selection matrix with the psi first-layer affine folded in:
    # row k<64 holds W1-scaled routing, row 64 holds b1 (paired with the
    # constant-ones row of xT), so z = W1*x + b1 comes out of the PE.
    sel_d = nc.dram_tensor("sel", [I + 1, CHUNKS * 128], F32R, kind="ExternalInput").ap()
    y_d = nc.dram_tensor("y", [BS, 1], F32, kind="ExternalOutput").ap()
    y_row = y_d.rearrange("b one -> one b")

    # x viewed so one DMA lands [128, 8, 64] per window:
    # xbm[p, t, d] = x[t*128 + p, d]
    x_bm = x_d.rearrange("(t p) d -> p t d", p=128)

    with tile.TileContext(nc) as tc, ExitStack() as ctx:
        const = ctx.enter_context(tc.tile_pool(name="const", bufs=1))
        hpool = ctx.enter_context(tc.tile_pool(name="h", bufs=3))
        zpool = ctx.enter_context(tc.tile_pool(name="z", bufs=3, space="PSUM"))
        ypool = ctx.enter_context(tc.tile_pool(name="yps", bufs=1, space="PSUM"))

        opool = ctx.enter_context(tc.tile_pool(name="osb", bufs=2))
        xT = const.tile([D + 1, BS], F32R)
        xbm = const.tile([128, CHUNKS * D], F32)
        ident = const.tile([128, 128], F32)
        mes = const.tile([128, CHUNKS * I], F32R)
        befr = const.tile([1, I], F32R)
        ones1 = const.tile([1, NS], F32R)
        wgb = const.tile([I + 1, 1], F32R)
        sls = const.tile([I + 1, CHUNKS * 128], F32R)
        # manual double buffer for hg so its ones-row survives reuse
        hgs = [
            const.tile([I + 1, W], F32R, name=f"hg{k}") for k in range(2)
        ]

        make_identity(nc, ident[:])
        nc.vector.memset(xT[D : D + 1, :].bitcast(F32), 1.0)
        nc.vector.memset(ones1[:].bitcast(F32), 1.0)
        for t in hgs:
            nc.vector.memset(t[I : I + 1, :].bitcast(F32), 1.0)
        nc.sync.dma_start(mes[:], me_d)
        nc.sync.dma_start(befr[:], be_d)
        nc.sync.dma_start(wgb[:], wg_d)
        nc.sync.dma_start(sls[:], sel_d)

        if reps > 1:
            loop_ctx = tc.For_i(0, reps, 1)
            loop_ctx.__enter__()
        for w in range(NW):
            # stage x window into SBUF batch-major, then PE-transpose to xT
            nc.sync.dma_start(
                xbm[:, ts(w, 512)].rearrange("p (t d) -> p t d", d=D),
                x_bm[:, ts(w, 8), :],
            )
            for h2 in range(2):
                tp = zpool.tile([128, W], F32, tag="z")
                for j in range(4):
                    nc.tensor.transpose(
                        tp[0:D, ts(j, 128)],
                        xbm[:, ds(w * 512 + (4 * h2 + j) * 64, 64)],
                        ident[:],
                    )
                nc.vector.tensor_copy(
                    xT[0:D, ds(w * W + h2 * 512, 512)], tp[0:D, 0:512]
                )

        # Software-pipelined main loop over 128 flattened (window, chunk)
        # steps.  Selection matmuls run LOOKAHEAD steps ahead of the tanh
        # they feed and are emitted before the main matmul of the current
        # step, so the PE never gates ScalarE:
        #   PE:      ... sel_{j+2}, main_j ...    (854 ns/step + slack)
        #   ScalarE: ... act_{j+1} ...            (1038 ns/step, bottleneck)
        steps = [(w, c) for w in range(NW) for c in range(CHUNKS)]
        LOOKAHEAD = 2
        ztiles = {}

        def emit_sel(j):
            w, c = steps[j]
            z = zpool.tile([128, W], F32, tag="z")
            ztiles[j] = z
            for s in range(W // NS):
                nc.tensor.matmul(
                    z[:, ts(s, NS)],
                    sls[:, ts(c, 128)],
                    xT[:, ds(w * W + s * NS, NS)],
                    start=True,
                    stop=True,
                )

        for j in range(LOOKAHEAD):
            emit_sel(j)

        ypsum = None
        for j, (w, c) in enumerate(steps):
            if c == 0:
                ypsum = ypool.tile([I, W], F32)
                # open the accumulation group with u = beff (outer product
                # of beff row with the ones row of xT)
                for s in range(W // NS):
                    nc.tensor.matmul(
                        ypsum[:, ts(s, NS)],
                        befr[:],
                        ones1[:],
                        start=True,
                        stop=False,
                    )
            z = ztiles.pop(j)
            h = hpool.tile([128, W], F32R)
            # pure tanh: z already carries W1*x + b1 from the selection matmul
            nc.scalar.activation(h[:], z[:], mybir.ActivationFunctionType.Tanh)
            if j + LOOKAHEAD < len(steps):
                emit_sel(j + LOOKAHEAD)
            for s in range(W // NS):
                nc.tensor.matmul(
                    ypsum[:, ts(s, NS)],
                    mes[:, ts(c, I)],
                    h[:, ts(s, NS)],
                    start=False,
                    stop=(c == CHUNKS - 1),
                )
            if c == CHUNKS - 1:
                hg = hgs[w % 2]
                nc.scalar.activation(
                    hg[0:I, :], ypsum[:], mybir.ActivationFunctionType.Tanh
                )
                # matvec with [Wg2; bg2] against hg (ones row adds bg2);
                # result lands in a zpool slot, DMA straight from PSUM
                mv = zpool.tile([128, W], F32, tag="z")
                for s in range(W // NS):
                    nc.tensor.matmul(
                        mv[0:1, ts(s, NS)],
                        wgb[:],
                        hg[:, ts(s, NS)],
                        start=True,
                        stop=True,
                    )
                osb = opool.tile([1, W], F32)
                nc.vector.tensor_copy(osb[:], mv[0:1, 0:W])
                nc.sync.dma_start(y_row[:, ts(w, W)], osb[:])
        if reps > 1:
            loop_ctx.__exit__(None, None, None)

    nc.compile()
    return nc


_PROGRAM_CACHE = {}


def _get_program(reps: int = 1):
    if reps not in _PROGRAM_CACHE:
        _PROGRAM_CACHE[reps] = _build_program(reps)
    return _PROGRAM_CACHE[reps]


def _round_f32r(a):
    """Round fp32 to the nearest value representable as bf16_hi + bf16_lo."""
    import ml_dtypes
    a = np.asarray(a, np.float32)
    hi = a.astype(ml_dtypes.bfloat16).astype(np.float32)
    lo = (a - hi).astype(ml_dtypes.bfloat16).astype(np.float32)
    return hi + lo


def _prepare_weight_maps(W1, b1, W2, b2, Wg1, bg1, Wg2, bg2):
    W1 = np.asarray(W1, np.float32)
    b1 = np.asarray(b1, np.float32)
    W2 = np.asarray(W2, np.float64)
    b2 = np.asarray(b2, np.float64)
    Wg1 = np.asarray(Wg1, np.float64)
    bg1 = np.asarray(bg1, np.float64)
    Wg2 = np.asarray(Wg2, np.float32)
    bg2 = np.asarray(bg2, np.float32)

    # Fold: Meff[(d,i), f] = sum_o W2[d,i,o] Wg1[d*I+o, f]
    Wg1r = Wg1.reshape(D, I, I)
    Meff = np.einsum("dio,dof->dif", W2, Wg1r).astype(np.float32)  # [D, I, I]
    beff = (bg1 + b2.reshape(-1) @ Wg1).astype(np.float32)  # [I]

    # Chunk layouts: chunk c covers d = 2c, 2c+1; partition p = (d_rel<<6)|i
    # meffc[p, c*I + f] = Meff[2c + (p>>6), p&63, f]
    meffc = (
        Meff.reshape(CHUNKS, 2 * I, I).transpose(1, 0, 2).reshape(128, CHUNKS * I)
    ).copy()
    # sel[k, c*128 + m]: k<64 routes xT row d=2c+(m>>6) to chunk-partition m,
    # scaled by W1[d, i(m)]; row 64 pairs with the ones-row of xT and adds
    # b1[d, i(m)], so z = W1*x + b1 directly from the PE.
    sel = np.zeros((I + 1, CHUNKS, 128), np.float32)
    for c in range(CHUNKS):
        sel[2 * c, c, 0:I] = W1[2 * c]
        sel[2 * c + 1, c, I:128] = W1[2 * c + 1]
        sel[I, c, 0:I] = b1[2 * c]
        sel[I, c, I:128] = b1[2 * c + 1]
    sel = sel.reshape(I + 1, CHUNKS * 128)
    wgb = np.concatenate([Wg2.reshape(I, 1), bg2.reshape(1, 1)], axis=0)
    return {
        "meffc": _round_f32r(np.ascontiguousarray(meffc, np.float32)),
        "befr": _round_f32r(beff.reshape(1, I)),
        "wgb": _round_f32r(wgb),
        "sel": _round_f32r(sel),
    }


def kernel(x, W1, b1, W2, b2, Wg1, bg1, Wg2, bg2, _trace=False):
    x = np.ascontiguousarray(np.asarray(x, np.float32))
    assert x.shape == (B_TOT, D)
    wmap = _prepare_weight_maps(W1, b1, W2, b2, Wg1, bg1, Wg2, bg2)
    nc = _get_program()
    in_maps = [
        {"x": np.ascontiguousarray(x[i * BS : (i + 1) * BS]), **wmap}
        for i in range(N_CORES)
    ]
    res = run_bass_kernel_spmd(nc, in_maps, list(range(N_CORES)), trace=_trace)
    y = np.concatenate([r["y"] for r in res.results], axis=0)
    if _trace:
        kernel.last_results = res
    return y.astype(np.float32)


# revision 19
# speedup vs baseline: 1.9894x; 1.0257x over previous
"""Trainium2 Bass kernel for nn_KolmogorovLayer (dense_mlp).

Math (reference):
    h   = tanh(x[:,:,None] * W1 + b1)              # [B, D, I]
    psi = einsum('bdi,dio->bdo', h, W2) + b2       # [B, D, I]
    hg  = tanh(psi.reshape(B, D*I) @ Wg1 + bg1)    # [B, I]
    out = hg @ Wg2 + bg2                           # [B, 1]

Because psi feeds a linear layer, fold W2 and Wg1 on the host:
    Meff[(d,i), f] = sum_o W2[d,i,o] * Wg1[d*I+o, f]
    beff[f]        = bg1[f] + b2.reshape(-1) @ Wg1
    u = h.reshape(B, D*I) @ Meff + beff;  out = tanh(u) @ Wg2 + bg2

The kernel runs a transposed pipeline per core (batch sharded 8 ways,
4096 rows/core), with EVERY affine folded into PE matmuls so that all
ScalarE activations are pure tanh (vector scale/bias operands cost
~1.2us per activation on real HW):
    - DMA x -> SBUF batch-major, PE-transpose to xT [64, 4096]; xT row 64
      is a constant-ones row.
    - per 128-row (d,i)-chunk: a K=65 selection matmul computes
      z = W1*x + b1 directly in PSUM (W1 in the selection rows, b1 against
      the ones row); ScalarE computes h = tanh(z), pure.
    - main contraction accumulates u in PSUM over 32 chunks; beff enters
      via a [1,64] x ones matmul that opens the accumulation group.
    - ScalarE tanh(u) -> hg (rows 0..63 of a tile whose row 64 is ones);
      a K=65 matvec with [Wg2; bg2] gives y in PSUM; DMA out from PSUM.

Engine budget per core per iteration (cost-model):  ScalarE 132x1038ns
(bottleneck), PE ~113us, PSUM 8 banks = z 3x2 + ypsum 2.
"""

import numpy as np
from contextlib import ExitStack

import concourse.bass as bass
import concourse.bacc as bacc
import concourse.mybir as mybir
import concourse.tile as tile
from concourse.bass import ts, ds
from concourse.bass_utils import run_bass_kernel_spmd
from concourse.masks import make_identity

F32 = mybir.dt.float32
F32R = mybir.dt.float32r

B_TOT, D, I = 32768, 64, 64
N_CORES = 8
BS = B_TOT // N_CORES          # 4096 rows per core
CHUNKS = D * I // 128          # 32 chunks of 128 (d,i) rows
W = 1024                       # batch window (PSUM-limited)
NW = BS // W                   # 4 windows
NS = 512                       # matmul moving-operand stream width
UNROLL = 8                     # batch passes per For_i iteration (reps mode)


def _build_program(reps: int = 1):
    nc = bacc.Bacc(
        "TRN2", target_bir_lowering=False, debug=False, num_devices=N_CORES
    )
    x_d = nc.dram_tensor("x", [BS, D], F32, kind="ExternalInput").ap()
    me_d = nc.dram_tensor("meffc", [128, CHUNKS * I], F32R, kind="ExternalInput").ap()
    be_d = nc.dram_tensor("befr", [1, I], F32R, kind="ExternalInput").ap()
    wg_d = nc.dram_tensor("wgb", [I + 1, 1], F32R, kind="ExternalInput").ap()
    # selection matrix with the psi first-layer affine folded in:
    # row k<64 holds W1-scaled routing, row 64 holds b1 (paired with the
    # constant-ones row of xT), so z = W1*x + b1 comes out of the PE.
    sel_d = nc.dram_tensor("sel", [I + 1, CHUNKS * 128], F32R, kind="ExternalInput").ap()
    y_d = nc.dram_tensor("y", [BS, 1], F32, kind="ExternalOutput").ap()
    y_row = y_d.rearrange("b one -> one b")

    # x viewed so one DMA lands [128, 8, 64] per window:
    # xbm[p, t, d] = x[t*128 + p, d]
    x_bm = x_d.rearrange("(t p) d -> p t d", p=128)

    with tile.TileContext(nc) as tc, ExitStack() as ctx:
        const = ctx.enter_context(tc.tile_pool(name="const", bufs=1))
        hpool = ctx.enter_context(tc.tile_pool(name="h", bufs=3))
        zpool = ctx.enter_context(tc.tile_pool(name="z", bufs=3, space="PSUM"))
        ypool = ctx.enter_context(tc.tile_pool(name="yps", bufs=1, space="PSUM"))

        opool = ctx.enter_context(tc.tile_pool(name="osb", bufs=2))
        xT = const.tile([D + 1, BS], F32R)
        xbm = const.tile([128, CHUNKS * D], F32)
        ident = const.tile([128, 128], F32)
        mes = const.tile([128, CHUNKS * I], F32R)
        befr = const.tile([1, I], F32R)
        ones1 = const.tile([1, NS], F32R)
        wgb = const.tile([I + 1, 1], F32R)
        sls = const.tile([I + 1, CHUNKS * 128], F32R)
        # manual double buffer for hg so its ones-row survives reuse
        hgs = [
            const.tile([I + 1, W], F32R, name=f"hg{k}") for k in range(2)
        ]

        make_identity(nc, ident[:])
        nc.vector.memset(xT[D : D + 1, :].bitcast(F32), 1.0)
        nc.vector.memset(ones1[:].bitcast(F32), 1.0)
        for t in hgs:
            nc.vector.memset(t[I : I + 1, :].bitcast(F32), 1.0)
        nc.sync.dma_start(mes[:], me_d)
        nc.sync.dma_start(befr[:], be_d)
        nc.sync.dma_start(wgb[:], wg_d)
        nc.sync.dma_start(sls[:], sel_d)

        # Software-pipelined pass over 128 flattened (window, chunk) steps.
        # Selection matmuls run LOOKAHEAD steps ahead of the tanh they feed
        # and are emitted before the main matmul of the current step, so the
        # PE never gates ScalarE:
        #   PE:      ... sel_{j+2}, main_j ...    (854 ns/step + slack)
        #   ScalarE: ... act_{j+1} ...            (1038 ns/step, bottleneck)
        steps = [(w, c) for w in range(NW) for c in range(CHUNKS)]
        LOOKAHEAD = 2
        ztiles = {}

        def emit_sel(j):
            w, c = steps[j]
            z = zpool.tile([128, W], F32, tag="z")
            ztiles[j] = z
            for s in range(W // NS):
                nc.tensor.matmul(
                    z[:, ts(s, NS)],
                    sls[:, ts(c, 128)],
                    xT[:, ds(w * W + s * NS, NS)],
                    start=True,
                    stop=True,
                )

        def emit_pass(it):
            for w in range(NW):
                # stage x window into SBUF batch-major, PE-transpose to xT
                nc.sync.dma_start(
                    xbm[:, ts(w, 512)].rearrange("p (t d) -> p t d", d=D),
                    x_bm[:, ts(w, 8), :],
                )
                for h2 in range(2):
                    tp = zpool.tile([128, W], F32, tag="z")
                    for j in range(4):
                        nc.tensor.transpose(
                            tp[0:D, ts(j, 128)],
                            xbm[:, ds(w * 512 + (4 * h2 + j) * 64, 64)],
                            ident[:],
                        )
                    nc.vector.tensor_copy(
                        xT[0:D, ds(w * W + h2 * 512, 512)], tp[0:D, 0:512]
                    )

            for j in range(LOOKAHEAD):
                emit_sel(j)

            ypsum = None
            for j, (w, c) in enumerate(steps):
                if c == 0:
                    ypsum = ypool.tile([I, W], F32)
                    # open the accumulation group with u = beff (outer
                    # product of beff row with a constant-ones row)
                    for s in range(W // NS):
                        nc.tensor.matmul(
                            ypsum[:, ts(s, NS)],
                            befr[:],
                            ones1[:],
                            start=True,
                            stop=False,
                        )
                z = ztiles.pop(j)
                h = hpool.tile([128, W], F32R)
                # pure tanh: z carries W1*x + b1 from the selection matmul
                nc.scalar.activation(
                    h[:], z[:], mybir.ActivationFunctionType.Tanh
                )
                if j + LOOKAHEAD < len(steps):
                    emit_sel(j + LOOKAHEAD)
                for s in range(W // NS):
                    nc.tensor.matmul(
                        ypsum[:, ts(s, NS)],
                        mes[:, ts(c, I)],
                        h[:, ts(s, NS)],
                        start=False,
                        stop=(c == CHUNKS - 1),
                    )
                if c == CHUNKS - 1:
                    hg = hgs[(it * NW + w) % 2]
                    nc.scalar.activation(
                        hg[0:I, :], ypsum[:], mybir.ActivationFunctionType.Tanh
                    )
                    # matvec with [Wg2; bg2] against hg (ones row adds bg2);
                    # result lands in a zpool slot, staged out via DVE
                    mv = zpool.tile([128, W], F32, tag="z")
                    for s in range(W // NS):
                        nc.tensor.matmul(
                            mv[0:1, ts(s, NS)],
                            wgb[:],
                            hg[:, ts(s, NS)],
                            start=True,
                            stop=True,
                        )
                    osb = opool.tile([1, W], F32)
                    nc.vector.tensor_copy(osb[:], mv[0:1, 0:W])
                    nc.sync.dma_start(y_row[:, ts(w, W)], osb[:])

        if reps > 1:
            # UNROLL passes per hardware-loop iteration: For_i drains all
            # engines at its back-edge (semaphore reset barrier), so fewer,
            # fatter iterations amortize the per-iteration pipeline drain.
            assert reps % UNROLL == 0, (reps, UNROLL)
            loop_ctx = tc.For_i(0, reps // UNROLL, 1)
            loop_ctx.__enter__()
            for it in range(UNROLL):
                emit_pass(it)
            loop_ctx.__exit__(None, None, None)
        else:
            emit_pass(0)

    nc.compile()
    return nc


_PROGRAM_CACHE = {}


def _get_program(reps: int = 1):
    if reps not in _PROGRAM_CACHE:
        _PROGRAM_CACHE[reps] = _build_program(reps)
    return _PROGRAM_CACHE[reps]


def _round_f32r(a):
    """Round fp32 to the nearest value representable as bf16_hi + bf16_lo."""
    import ml_dtypes
    a = np.asarray(a, np.float32)
    hi = a.astype(ml_dtypes.bfloat16).astype(np.float32)
    lo = (a - hi).astype(ml_dtypes.bfloat16).astype(np.float32)
    return hi + lo


def _prepare_weight_maps(W1, b1, W2, b2, Wg1, bg1, Wg2, bg2):
    W1 = np.asarray(W1, np.float32)
    b1 = np.asarray(b1, np.float32)
    W2 = np.asarray(W2, np.float64)
    b2 = np.asarray(b2, np.float64)
    Wg1 = np.asarray(Wg1, np.float64)
    bg1 = np.asarray(bg1, np.float64)
    Wg2 = np.asarray(Wg2, np.float32)
    bg2 = np.asarray(bg2, np.float32)

    # Fold: Meff[(d,i), f] = sum_o W2[d,i,o] Wg1[d*I+o, f]
    Wg1r = Wg1.reshape(D, I, I)
    Meff = np.einsum("dio,dof->dif", W2, Wg1r).astype(np.float32)  # [D, I, I]
    beff = (bg1 + b2.reshape(-1) @ Wg1).astype(np.float32)  # [I]

    # Chunk layouts: chunk c covers d = 2c, 2c+1; partition p = (d_rel<<6)|i
    # meffc[p, c*I + f] = Meff[2c + (p>>6), p&63, f]
    meffc = (
        Meff.reshape(CHUNKS, 2 * I, I).transpose(1, 0, 2).reshape(128, CHUNKS * I)
    ).copy()
    # sel[k, c*128 + m]: k<64 routes xT row d=2c+(m>>6) to chunk-partition m,
    # scaled by W1[d, i(m)]; row 64 pairs with the ones-row of xT and adds
    # b1[d, i(m)], so z = W1*x + b1 directly from the PE.
    sel = np.zeros((I + 1, CHUNKS, 128), np.float32)
    for c in range(CHUNKS):
        sel[2 * c, c, 0:I] = W1[2 * c]
        sel[2 * c + 1, c, I:128] = W1[2 * c + 1]
        sel[I, c, 0:I] = b1[2 * c]
        sel[I, c, I:128] = b1[2 * c + 1]
    sel = sel.reshape(I + 1, CHUNKS * 128)
    wgb = np.concatenate([Wg2.reshape(I, 1), bg2.reshape(1, 1)], axis=0)
    return {
        "meffc": _round_f32r(np.ascontiguousarray(meffc, np.float32)),
        "befr": _round_f32r(beff.reshape(1, I)),
        "wgb": _round_f32r(wgb),
        "sel": _round_f32r(sel),
    }


def kernel(x, W1, b1, W2, b2, Wg1, bg1, Wg2, bg2, _trace=False):
    x = np.ascontiguousarray(np.asarray(x, np.float32))
    assert x.shape == (B_TOT, D)
    wmap = _prepare_weight_maps(W1, b1, W2, b2, Wg1, bg1, Wg2, bg2)
    nc = _get_program()
    in_maps = [
        {"x": np.ascontiguousarray(x[i * BS : (i + 1) * BS]), **wmap}
        for i in range(N_CORES)
    ]
    res = run_bass_kernel_spmd(nc, in_maps, list(range(N_CORES)), trace=_trace)
    y = np.concatenate([r["y"] for r in res.results], axis=0)
    if _trace:
        kernel.last_results = res
    return y.astype(np.float32)
